# revision 6
# baseline (speedup 1.0000x reference)
"""Expert-parallel Trainium2 Bass kernel for DeepEquiCategorySpecificMLP.

Routing strategy (host side): tokens are sorted by cat_id; core c receives
all tokens of category c (padded to a fixed PAD) plus that category's
weight stack. All compute (input LN, 5 matmuls, gated MLP, 3 more LNs,
residual) runs on-device in a feature-major layout ([feature, token]), so
every matmul consumes activations directly as the moving operand with the
weight stack as the stationary operand (out = W.T @ actT) and no on-device
transposes are needed. LayerNorm is over the feature axis = partition axis:
sums are computed on the TensorEngine (ones-vector matmuls accumulating in
PSUM), per-token scale/shift rows are broadcast across partitions on
GPSIMD, and applied on the VectorEngine.
"""

import numpy as np
from contextlib import ExitStack

N_CORES = 8
D = 256
H = 1024
EPS = 1e-5
PAD_MIN = 288  # >= max per-category count (283 at seed 0); >=256 keeps f32r matmuls full-rate

# Experiment knobs
MM_DTYPE = "f32r"  # "f32r" | "bf16" | "f32"
BCAST = "gpsimd"   # "gpsimd" | "pe"

_cache = {}


def _build(PAD, center_only_gln):
    import concourse.bass as bass
    import concourse.tile as tile
    from concourse import bacc, mybir

    f32 = mybir.dt.float32
    f32r = mybir.dt.float32r
    mmdt = f32r if MM_DTYPE == "f32r" else f32
    AF = mybir.ActivationFunctionType
    ALU = mybir.AluOpType
    KD, KH = D // 128, H // 128

    nc = bacc.Bacc("TRN2", target_bir_lowering=False, debug=False,
                   num_devices=N_CORES)

    xT_d = nc.dram_tensor("xT", [D, PAD], mmdt, kind="ExternalInput")
    w0_d = nc.dram_tensor("W0", [D, H], mmdt, kind="ExternalInput")
    wm_d = nc.dram_tensor("Wm", [H, H], mmdt, kind="ExternalInput")
    wg_d = nc.dram_tensor("Wg", [H, H], mmdt, kind="ExternalInput")
    wog_d = nc.dram_tensor("Wog", [H, H], mmdt, kind="ExternalInput")
    w2_d = nc.dram_tensor("W2", [H, D], mmdt, kind="ExternalInput")
    b0_d = nc.dram_tensor("b0", [H], f32, kind="ExternalInput")
    bm_d = nc.dram_tensor("bm", [H], f32, kind="ExternalInput")
    bg_d = nc.dram_tensor("bg", [H], f32, kind="ExternalInput")
    bog_d = nc.dram_tensor("bog", [H], f32, kind="ExternalInput")
    b2_d = nc.dram_tensor("b2", [D], f32, kind="ExternalInput")
    out_d = nc.dram_tensor("outT", [D, PAD], f32, kind="ExternalOutput")

    with ExitStack() as ctx:
        tc = ctx.enter_context(tile.TileContext(nc))
        wp = ctx.enter_context(tc.tile_pool(name="w", bufs=1))
        ap_ = ctx.enter_context(tc.tile_pool(name="a", bufs=1))
        sqp = ctx.enter_context(tc.tile_pool(name="sq", bufs=3))
        stp = ctx.enter_context(tc.tile_pool(name="st", bufs=2))
        pmm = ctx.enter_context(
            tc.tile_pool(name="pmm", bufs=4, space=bass.MemorySpace.PSUM))
        pst = ctx.enter_context(
            tc.tile_pool(name="pst", bufs=2, space=bass.MemorySpace.PSUM))

        def r(apx):
            return apx

        def load_w(dram, K, mfree, tagp):
            tiles = []
            for k in range(K):
                t = wp.tile([128, mfree], mmdt, tag=f"{tagp}{k}", name=f"{tagp}{k}")
                nc.sync.dma_start(t[:], dram.ap()[k * 128:(k + 1) * 128, :])
                tiles.append(t)
            return tiles

        def load_b(dram, K, tag):
            t = wp.tile([128, K], f32, tag=tag, name=tag)
            nc.sync.dma_start(t[:], dram.ap().rearrange("(j p) -> p j", p=128))
            return t

        # DMA issue order == consumption order so compute tracks the stream.
        xT = load_w(xT_d, KD, PAD, "xT")
        w0 = load_w(w0_d, KD, H, "w0")
        b0t = load_b(b0_d, KH, "b0")
        bmt = load_b(bm_d, KH, "bm")
        bgt = load_b(bg_d, KH, "bg")
        bogt = load_b(bog_d, KH, "bog")
        b2t = load_b(b2_d, KD, "b2")
        wm = load_w(wm_d, KH, H, "wm")
        wg = load_w(wg_d, KH, H, "wg")
        wog = load_w(wog_d, KH, H, "wog")
        w2 = load_w(w2_d, KH, D, "w2")

        onesf = wp.tile([128, 1], f32, tag="onesf", name="onesf")
        nc.vector.memset(onesf[:], 1.0)
        onesc = wp.tile([128, 1], mmdt, tag="ones", name="ones")
        nc.vector.tensor_copy(onesc[:], onesf[:])
        if BCAST == "pe":
            onesr = wp.tile([1, 128], mmdt, tag="onesr", name="onesr")
            nc.vector.tensor_copy(onesr[:], onesf[:1, :].broadcast_to([1, 128]))
        eps_t = {}
        for F in (D, H):
            t = wp.tile([1, 1], f32, tag=f"eps{F}", name=f"eps{F}")
            nc.vector.memset(t[:], float(F) * float(F) * EPS)
            eps_t[F] = t

        def stats_sum(x_tiles):
            s = pst.tile([1, PAD], f32, tag="st", name="stat")
            K = len(x_tiles)
            for k in range(K):
                nc.tensor.matmul(s[:], r(onesc[:]), r(x_tiles[k][:]),
                                 start=(k == 0), stop=(k == K - 1))
            return s

        def stats_sumsq(x_tiles):
            s = pst.tile([1, PAD], f32, tag="st", name="stat")
            K = len(x_tiles)
            for k in range(K):
                sqt = sqp.tile([128, PAD], mmdt, tag="sqt", name="sqt")
                nc.vector.tensor_mul(sqt[:], x_tiles[k][:], x_tiles[k][:])
                nc.tensor.matmul(s[:], r(onesc[:]), r(sqt[:]),
                                 start=(k == 0), stop=(k == K - 1))
            return s

        def bcast(src_row, tag, btag="bcA"):
            if BCAST == "gpsimd":
                b = ap_.tile([128, PAD], f32, tag=btag, name=tag, bufs=2)
                nc.gpsimd.partition_broadcast(b[:], src_row[:])
            else:
                b = pmm.tile([128, PAD], f32, tag="bc", name="bc")
                nc.tensor.matmul(b[:], r(onesr[:]), r(src_row[:]),
                                 start=True, stop=True)
            return b

        def ln_full(x_tiles, F, pref):
            """Stats for LN over the partition (feature) axis.

            Returns (rinv_b, B_b) with normalized = (x*F)*rinv_b + B_b,
            rinv = 1/sqrt(F*s2 - s1^2 + F^2*eps), B = -s1*rinv.
            """
            s1 = stats_sum(x_tiles)
            s2 = stats_sumsq(x_tiles)
            s1s = stp.tile([1, PAD], f32, tag="st_s1", name=f"{pref}s1")
            nc.vector.tensor_copy(s1s[:], s1[:])
            s2s = stp.tile([1, PAD], f32, tag="st_s2", name=f"{pref}s2")
            nc.vector.tensor_copy(s2s[:], s2[:])
            t1 = stp.tile([1, PAD], f32, tag="st_t1", name=f"{pref}t1")
            nc.vector.tensor_mul(t1[:], s1s[:], s1s[:])
            u = stp.tile([1, PAD], f32, tag="st_u", name=f"{pref}u")
            nc.vector.scalar_tensor_tensor(u[:], s2s[:], float(F), t1[:],
                                           op0=ALU.mult, op1=ALU.subtract)
            sqv = stp.tile([1, PAD], f32, tag="st_sv", name=f"{pref}sv")
            nc.scalar.activation(sqv[:], u[:], AF.Sqrt, bias=eps_t[F][:])
            rinv = stp.tile([1, PAD], f32, tag="st_ri", name=f"{pref}ri")
            nc.vector.reciprocal(rinv[:], sqv[:])
            Bs = stp.tile([1, PAD], f32, tag="st_Bs", name=f"{pref}Bs")
            nc.vector.scalar_tensor_tensor(Bs[:], s1s[:], -1.0, rinv[:],
                                           op0=ALU.mult, op1=ALU.mult)
            return bcast(rinv, f"{pref}A", "bcA"), bcast(Bs, f"{pref}B", "bcB")

        def apply_full(x_k, out_k, F, Ab, Bb):
            nc.vector.scalar_tensor_tensor(out_k[:], x_k[:], float(F), Ab[:],
                                           op0=ALU.mult, op1=ALU.mult)
            nc.vector.tensor_add(out_k[:], out_k[:], Bb[:])

        def mm_layer(wtiles, atiles, K, MT, mgroup, evac):
            outs = []
            for g0 in range(0, MT, mgroup):
                ms = list(range(g0, min(g0 + mgroup, MT)))
                pss = [pmm.tile([128, PAD], f32, tag="mmps", name="mmps") for _ in ms]
                for k in range(K):
                    for i, m in enumerate(ms):
                        nc.tensor.matmul(
                            pss[i][:],
                            r(wtiles[k][:, m * 128:(m + 1) * 128]),
                            r(atiles[k][:]),
                            start=(k == 0), stop=(k == K - 1))
                for i, m in enumerate(ms):
                    outs.append(evac(m, pss[i]))
            return outs

        def evac_act(func, bias_tile, tagp):
            def f(m, ps):
                t = ap_.tile([128, PAD], mmdt, tag=f"{tagp}{m}", name=f"{tagp}{m}")
                nc.scalar.activation(t[:], ps[:], func,
                                     bias=bias_tile[:, m:m + 1])
                return t
            return f

        # ---- input LN over D ----
        Ab, Bb = ln_full(xT, D, "iln")
        xn = []
        for k in range(KD):
            t = ap_.tile([128, PAD], mmdt, tag=f"xn{k}", name=f"xn{k}")
            apply_full(xT[k], t, D, Ab, Bb)
            xn.append(t)

        # ---- h = relu(xn @ W0 + b0) ----
        h = mm_layer(w0, xn, KD, KH, 4, evac_act(AF.Relu, b0t, "h"))

        # ---- main/gate, gated = main * sigmoid(gate) ----
        mainT = mm_layer(wm, h, KH, KH, 4, evac_act(AF.Identity, bmt, "mn"))
        sigT = mm_layer(wg, h, KH, KH, 4, evac_act(AF.Sigmoid, bgt, "sg"))
        for k in range(KH):
            nc.vector.tensor_mul(mainT[k][:], mainT[k][:], sigT[k][:])

        # ---- g = LN(gated): when bog == 0 the per-token scale washes out in
        # the next LN, so only centering is required.
        if center_only_gln:
            s1 = stats_sum(mainT)
            s1s = stp.tile([1, PAD], f32, tag="st_s1", name="gls1")
            nc.vector.tensor_copy(s1s[:], s1[:])
            Bs = stp.tile([1, PAD], f32, tag="st_Bs", name="glBs")
            nc.vector.tensor_scalar_mul(Bs[:], s1s[:], -1.0 / float(H))
            Bb1 = bcast(Bs, "glB", "bcB")
            for k in range(KH):
                nc.vector.tensor_add(mainT[k][:], mainT[k][:], Bb1[:])
        else:
            Ab1, Bb1 = ln_full(mainT, H, "gln")
            for k in range(KH):
                apply_full(mainT[k], mainT[k], H, Ab1, Bb1)

        # ---- h2 = LN(g @ Wog + bog) ----
        h2 = mm_layer(wog, mainT, KH, KH, 4, evac_act(AF.Identity, bogt, "h2"))
        Ab2, Bb2 = ln_full(h2, H, "hln")
        for k in range(KH):
            apply_full(h2[k], h2[k], H, Ab2, Bb2)

        # ---- y = h2 @ W2 + b2 ; out = LN(y + 0.1 x) ----
        y = mm_layer(w2, h2, KH, KD, 2, evac_act(AF.Identity, b2t, "y"))
        opre = []
        for k in range(KD):
            t = ap_.tile([128, PAD], mmdt, tag=f"op{k}", name=f"op{k}")
            nc.vector.scalar_tensor_tensor(t[:], xT[k][:], 0.1, y[k][:],
                                           op0=ALU.mult, op1=ALU.add)
            opre.append(t)
        Ab3, Bb3 = ln_full(opre, D, "oln")
        for k in range(KD):
            ot = ap_.tile([128, PAD], f32, tag=f"ot{k}", name=f"ot{k}")
            apply_full(opre[k], ot, D, Ab3, Bb3)
            nc.sync.dma_start(out_d.ap()[k * 128:(k + 1) * 128, :], ot[:])

    nc.compile()
    return nc


def _get_nc(PAD, center_only_gln):
    key = (PAD, center_only_gln, MM_DTYPE, BCAST)
    if key not in _cache:
        _cache[key] = _build(PAD, center_only_gln)
    return _cache[key]


def _prep(x, cat_ids, W0, b0, Wm, bm, Wg, bg, Wog, bog, W2, b2):
    x = np.ascontiguousarray(np.asarray(x, dtype=np.float32))
    cid = np.asarray(cat_ids).astype(np.int64).ravel()
    counts = np.bincount(cid, minlength=N_CORES)
    PAD = int(max(PAD_MIN, ((counts.max() + 31) // 32) * 32))
    order = np.argsort(cid, kind="stable")
    starts = np.zeros(N_CORES + 1, np.int64)
    starts[1:] = np.cumsum(counts)

    def f32c(a):
        return np.ascontiguousarray(np.asarray(a, dtype=np.float32))

    in_maps = []
    for c in range(N_CORES):
        ids = order[starts[c]:starts[c + 1]]
        xc = np.zeros((PAD, D), np.float32)
        xc[:len(ids)] = x[ids]
        in_maps.append({
            "xT": np.ascontiguousarray(xc.T),
            "W0": f32c(W0[c]), "Wm": f32c(Wm[c]), "Wg": f32c(Wg[c]),
            "Wog": f32c(Wog[c]), "W2": f32c(W2[c]),
            "b0": f32c(b0[c]), "bm": f32c(bm[c]), "bg": f32c(bg[c]),
            "bog": f32c(bog[c]), "b2": f32c(b2[c]),
        })
    center_only = not np.any(np.asarray(bog))
    return in_maps, order, starts, PAD, center_only, x.shape[0]


def kernel(x, cat_ids, W0, b0, Wm, bm, Wg, bg, Wog, bog, W2, b2, **run_kwargs):
    from concourse.bass_utils import run_bass_kernel_spmd

    in_maps, order, starts, PAD, center_only, N = _prep(
        x, cat_ids, W0, b0, Wm, bm, Wg, bg, Wog, bog, W2, b2)
    nc = _get_nc(PAD, center_only)
    res = run_bass_kernel_spmd(nc, in_maps, core_ids=list(range(N_CORES)),
                               **run_kwargs)
    out = np.zeros((N, D), np.float32)
    for c in range(N_CORES):
        ids = order[starts[c]:starts[c + 1]]
        out[ids] = res.results[c]["outT"].T[:len(ids)]
    if run_kwargs:
        kernel.last_results = res
    return out
